# revision 72
# baseline (speedup 1.0000x reference)
"""Causal single-head attention (B=4, T=4096, C=1024, H=64) on 8 TRN2 cores.

Sharding: 2 cores per batch element, causal-balanced interleaved query
blocks of 256: half 0 owns even 256-blocks, half 1 owns odd ones.  Core
slot j processes local query block j (global block 2j+half) against key
tiles 0..4j+4.  Both halves run ONE identical SPMD program; asymmetry is
data-only (tau mask column, q-select scalars).

Per-core pipeline (all matmuls f16, cost = out-free-dim cycles):
  - [Q^T|K^T] projection for ALL T columns (Q rides along in the same
    128-wide output, so no separate half-rate Q pass).
  - V projected directly into [keys, H] layout by swapping matmul
    operand roles (lhsT = X^T chunk, rhs = Wv): output free dim is 64,
    half the cost of a [H, keys]-layout projection, and no transposes.
    V of the last two blocks is deferred into their own (Act-bound)
    rounds as PE filler.
  - Scores S^T: K^T-stationary matmuls (flat [64, T] K^T, all at
    tile_position (0,0) — tile_position row offsets with sub-bank PSUM
    outputs break on HW) into quad PSUM tiles [128, 4tile, 256q]; one
    Exp per quad on Activation (the saturating engine in the back
    half); ONE tensor_tensor mask multiply on the diagonal quad per
    slot against a precomputed constant mask (tau[kp] = kp - 256*half
    works for every slot and tile via an iota with -128/tile steps).
  - O accumulated P-stationary: lhsT = P chunk [128k, 128q], rhs =
    V tile [128k, 65] (ones column -> softmax denominator), so each
    accumulation matmul has free dim 65 instead of 256.  The two
    query-half chains accumulate in SEPARATE PSUM banks (interleaved
    accumulation groups within one bank corrupt on HW) interleaved
    into the score stream; each slot's trailing quads spill into the
    next round so PE never stalls on the slot's last exps.  Output
    lands directly in [query, H+1] layout: no transpose; the host
    divides by the denominator column and scatters.
"""

import os

import numpy as np

import concourse.bacc as bacc
import concourse.mybir as mybir
import concourse.tile as tile
from concourse.bass_utils import run_bass_kernel_spmd

B, T, C, H = 4, 4096, 1024, 64
NCORES = 8
QB = 256  # query block (one slot's queries)
KT = 128  # key tile
NSLOT = 8  # slots per core
CCH = C // 128  # contraction chunks
NQ = NSLOT * QB  # queries per core (2048)
F32 = mybir.dt.float32
F16 = mybir.dt.float16
I32 = mybir.dt.int32
XD = F16
XNP = np.float16

_PROGRAM = None


def _build_program():
    nc = bacc.Bacc(None, target_bir_lowering=False, debug=False)

    debug = bool(os.environ.get("KERNEL_DEBUG"))
    level = int(os.environ.get("KERNEL_LEVEL", "4"))
    xt = nc.dram_tensor("xt", [C, T], XD, kind="ExternalInput")
    wqk = nc.dram_tensor("wqk", [128, CCH * 128], XD, kind="ExternalInput")
    wv = nc.dram_tensor("wv", [128, CCH * 64], XD, kind="ExternalInput")
    tau = nc.dram_tensor("tau", [128, 1], F32, kind="ExternalInput")
    sel = nc.dram_tensor("sel", [64, 2], F32, kind="ExternalInput")
    o = nc.dram_tensor("o", [NQ, 65], F32, kind="ExternalOutput")

    xt_r = xt.rearrange("(n p) t -> p n t", p=128)  # [128, 8, T]
    wqk_r = wqk.rearrange("p (n m) -> p n m", m=128)  # [128, 8, 128]
    wv_r = wv.rearrange("p (n m) -> p n m", m=64)  # [128, 8, 64]
    # o row = 256*slot + 128*u + p
    o_r = o.rearrange("(s u p) c -> p s u c", p=128, u=2)  # [128, 8, 2, 65]

    with tile.TileContext(nc) as tc:
        with (
            tc.tile_pool(name="const", bufs=1) as const_pool,
            tc.tile_pool(name="big", bufs=1) as big_pool,
            tc.tile_pool(name="xin", bufs=3) as xin_pool,
            tc.tile_pool(name="q", bufs=3) as q_pool,
            tc.tile_pool(name="p", bufs=10) as p_pool,
            tc.tile_pool(name="outp", bufs=16) as out_pool,
            tc.tile_pool(name="ps_qk", bufs=1, space="PSUM") as ps_qk,
            tc.tile_pool(name="ps_st", bufs=2, space="PSUM") as ps_st,
            tc.tile_pool(name="ps_v", bufs=1, space="PSUM") as ps_v,
            tc.tile_pool(name="ps_o", bufs=1, space="PSUM") as ps_o,
        ):
            # ---- constants ----
            # weights/consts go on the Act/DVE HWDGE queues so the first X
            # block owns the SP queue from t=0
            wqk_s = const_pool.tile([128, CCH, 128], XD)
            nc.scalar.dma_start(wqk_s[:, 0:2], wqk_r[:, 0:2])
            nc.scalar.dma_start(wqk_s[:, 2:8], wqk_r[:, 2:8])
            wv_s = const_pool.tile([128, CCH, 64], XD)
            nc.scalar.dma_start(wv_s[:], wv_r)
            tau_s = const_pool.tile([128, 1], F32)
            nc.scalar.dma_start(tau_s[:], tau[:])
            sel_s = const_pool.tile([64, 2], F32)
            nc.scalar.dma_start(sel_s[:], sel[:])

            warm_sb = const_pool.tile([128, 128], XD)
            nc.vector.memset(warm_sb[:], 0.0)

            # iota[kp, i, qf] = qf - 128*i
            iota_i = const_pool.tile([128, 4, QB], I32)
            nc.gpsimd.iota(
                iota_i[:],
                pattern=[[-KT, 4], [1, QB]],
                base=0,
                channel_multiplier=0,
            )
            iota_f = const_pool.tile([128, 4, QB], XD)
            nc.vector.tensor_copy(iota_f[:], iota_i[:])
            # diagonal mask (slot-independent): mask = (iota >= tau)
            mask_s = const_pool.tile([128, 4, QB], XD)
            nc.vector.tensor_scalar(
                mask_s[:],
                iota_f[:],
                tau_s[:, 0:1],
                None,
                op0=mybir.AluOpType.is_ge,
            )

            # warm the Exp table while the first DMAs run
            dummy = const_pool.tile([64, 1], F32)
            nc.scalar.activation(
                dummy[:], sel_s[:, 0:1], mybir.ActivationFunctionType.Exp
            )

            # PE p-state warm-up: the tensor engine clock only reaches full
            # speed after 3us of continuous execution, and the head is
            # DMA-gated anyway.  Dummy matmuls on a zeroed tile (into the
            # not-yet-used o PSUM bank) ramp the clock and fill DMA waits.
            warm_ps = ps_o.tile([128, 2, 512], F32, tag="ps_o")

            def warm(n):
                for _ in range(n):
                    nc.tensor.matmul(
                        warm_ps[0:64, 0, 0:64],
                        warm_sb[:, 0:64],
                        warm_sb[:, 0:64],
                        start=True,
                        stop=True,
                        skip_group_check=True,
                    )

            # ---- persistent activations ----
            kT_s = big_pool.tile([64, T], XD)  # K^T, flat
            q_full = big_pool.tile([64, T], XD)  # Q^T, all T columns
            v_s = big_pool.tile([128, T // KT, 65], XD)  # V rows + ones col
            ones_sb = const_pool.tile([128, T // KT, 1], F32)
            nc.vector.memset(ones_sb[:], 1.0)
            nc.vector.tensor_copy(v_s[:, :, 64:65], ones_sb[:])

            def dma_block(b, pieces=2):
                xt_t = xin_pool.tile([128, CCH, 512], XD, tag="xin")
                n = CCH // pieces
                for i in range(pieces):
                    nc.sync.dma_start(
                        xt_t[:, i * n : (i + 1) * n],
                        xt_r[:, i * n : (i + 1) * n, b * 512 : (b + 1) * 512],
                    )
                return xt_t

            def proj_qk(b, xt_t, cis=range(CCH)):
                """[Q^T|K^T] for columns [512b, 512b+512)."""
                if 0 in cis:
                    qk_ps = ps_qk.tile([128, 512], F32, tag="ps_qk")
                else:
                    qk_ps = None
                for ci in cis:
                    nc.tensor.matmul(
                        proj_qk.ps[:] if qk_ps is None else qk_ps[:],
                        wqk_s[:, ci],
                        xt_t[:, ci],
                        start=ci == 0,
                        stop=ci == CCH - 1,
                    )
                if qk_ps is not None:
                    proj_qk.ps = qk_ps
                return proj_qk.ps

            def proj_qk_copies(b, qk_ps):
                nc.vector.tensor_copy(
                    kT_s[:, b * 512 : (b + 1) * 512], qk_ps[64:128, :]
                )
                nc.vector.tensor_copy(q_full[:, b * 512 : (b + 1) * 512], qk_ps[0:64, :])

            def proj_v(b, xt_t, tiles, v_ps):
                """V[k, h] direct for k-tiles `tiles` (subset of 4b..4b+3)."""
                for t in tiles:
                    lt = t - 4 * b
                    for ci in range(CCH):
                        nc.tensor.matmul(
                            v_ps[:, lt],
                            xt_t[:, ci, lt * KT : (lt + 1) * KT],
                            wv_s[:, ci],
                            start=ci == 0,
                            stop=ci == CCH - 1,
                        )

            def proj_v_copy(b, v_ps):
                nc.vector.tensor_copy(v_s[:, 4 * b : 4 * b + 4, 0:64], v_ps[:])

            def qsel(j):
                """Select this half's query block for slot j (data-driven)."""
                q_sb = q_pool.tile([64, QB], XD, tag="qslot")
                nc.vector.tensor_scalar_mul(
                    q_sb[:],
                    q_full[:, j * 512 : j * 512 + QB],
                    sel_s[:, 0:1],
                )
                nc.vector.scalar_tensor_tensor(
                    q_sb[:],
                    q_full[:, j * 512 + QB : (j + 1) * 512],
                    sel_s[:, 1:2],
                    q_sb[:],
                    mybir.AluOpType.mult,
                    mybir.AluOpType.add,
                )
                return q_sb

            def score_quad(j, qd, q_sb):
                """Tiles 4qd..4qd+3 vs slot j's queries."""
                st_ps = ps_st.tile([128, 4, QB], F32, tag="ps_st")
                for i in range(4):
                    nc.tensor.matmul(
                        st_ps[:, i],
                        kT_s[:, (4 * qd + i) * KT : (4 * qd + i + 1) * KT],
                        q_sb[:],
                        start=True,
                        stop=True,
                    )
                return st_ps

            def post_quad(j, qd, st_ps):
                p_sb = p_pool.tile([128, 4, QB], XD, tag="p")
                nc.scalar.activation(
                    p_sb[:],
                    st_ps[:],
                    mybir.ActivationFunctionType.Exp,
                    scale=float(H) ** -0.5,
                )
                if qd == j:  # diagonal quad: mask
                    nc.vector.tensor_tensor(
                        p_sb[:], p_sb[:], mask_s[:], mybir.AluOpType.mult
                    )
                return p_sb

            def o_part(j, qd, u, p_sb, o_ps, first, last):
                """Accumulate quad `qd` into query-half u's chain (own bank)."""
                for i in range(4):
                    nc.tensor.matmul(
                        o_ps[:, u, 0:65],
                        p_sb[:, i, u * KT : (u + 1) * KT],
                        v_s[:, 4 * qd + i, :],
                        start=first and i == 0,
                        stop=last and i == 3,
                        skip_group_check=True,
                    )

            pending_stores = []

            def store_half(j, u, o_ps):
                o_sb = out_pool.tile([128, 65], F32, tag="o_sb")
                nc.vector.tensor_copy(o_sb[:], o_ps[:, u, 0:65])
                if j < 6:  # keep the saturated X wire free; flush in round 7
                    pending_stores.append(
                        lambda j=j, u=u, o_sb=o_sb: nc.sync.dma_start(
                            o_r[:, j, u], o_sb[:]
                        )
                    )
                else:
                    nc.sync.dma_start(o_r[:, j, u], o_sb[:])

            # ---- main pipeline ----
            # (warm-ups write the o bank, which slot 0's deferred o parts
            # only touch from round 1 on — never warm after round 0)
            xt_tiles = {0: dma_block(0, pieces=4), 1: dma_block(1)}
            warm(56)
            qk0 = None
            for i in range(4):
                qk0 = proj_qk(0, xt_tiles[0], range(2 * i, 2 * i + 2))
                warm(6)
            proj_qk_copies(0, qk0)
            vps0 = ps_v.tile([128, 4, 64], F32, tag="ps_v")
            for t in range(4):
                proj_v(0, xt_tiles[0], [t], vps0)
                warm(4)
            proj_v_copy(0, vps0)
            q_next = qsel(0)
            warm(8)

            deferred = []
            for j in range(NSLOT):
                if j + 2 < NSLOT:
                    xt_tiles[j + 2] = dma_block(j + 2)
                q_sb = q_next
                o_ps = ps_o.tile([128, 2, 512], F32, tag="ps_o")
                sq = (
                    (lambda j, qd, q_sb: post_quad(j, qd, score_quad(j, qd, q_sb)))
                    if level >= 2
                    else (lambda j, qd, q_sb: None)
                )
                # diagonal quad early (its exp+mask clear the Act queue well
                # before its o matmuls) but not first (so the o chains, which
                # start with order[0], never head-of-line block on the mask).
                # In deferred-V rounds it must come late enough that its V
                # tiles (projected as filler in THIS round) precede it.
                if j < 2:
                    order = list(range(j + 1))
                elif j < 6:
                    order = [0, 1, j] + list(range(2, j))
                else:
                    order = [q for q in range(j + 1) if q != j]
                    order.insert(j - 1, j)
                ps = {}

                def emit_o(idx, j=j, order=order, ps=ps, o_ps=o_ps):
                    if level < 3:
                        return
                    qd = order[idx]
                    for u in range(2):
                        o_part(j, qd, u, ps[qd], o_ps, idx == 0, idx == j)
                    if level >= 4 and idx == j:
                        for u in range(2):
                            store_half(j, u, o_ps)

                if j == NSLOT - 1:
                    for f in pending_stores:
                        f()
                # score quads staggered two ahead of o accumulation
                for qd in order[0 : min(2, j + 1)]:
                    ps[qd] = sq(j, qd, q_sb)
                # previous slot's trailing o work runs here, after this
                # slot's first scores are already feeding Activation
                for f in deferred:
                    f()
                deferred = []
                # projection of the next block runs as PE filler spread
                # evenly between this slot's (Act-gated) score quads; V of
                # the last blocks is deferred into their own round
                pe_filler = []
                if j >= 6:  # deferred V for block j (this round's queries)
                    vd = ps_v.tile([128, 4, 64], F32, tag="ps_v")
                    for t in range(4 * j, 4 * j + 4):
                        pe_filler.append(
                            lambda t=t: proj_v(j, xt_tiles[j], [t], vd)
                        )
                    pe_filler.append(lambda: proj_v_copy(j, vd))
                if j + 1 < NSLOT:
                    xt_n = xt_tiles[j + 1]
                    if j == 0:
                        pe_filler.append(lambda: warm(12))
                    pe_filler.append(lambda: proj_qk(j + 1, xt_n, range(0, 4)))

                    def qk_rest():
                        qk = proj_qk(j + 1, xt_n, range(4, CCH))
                        proj_qk_copies(j + 1, qk)
                        nonlocal q_next
                        q_next = qsel(j + 1)

                    pe_filler.append(qk_rest)
                    if j + 1 < 6:  # V for next block, unless deferred
                        vps = ps_v.tile([128, 4, 64], F32, tag="ps_v")
                        for t in range(4 * j + 4, 4 * j + 8):
                            pe_filler.append(
                                lambda t=t: proj_v(j + 1, xt_n, [t], vps)
                            )
                        pe_filler.append(lambda: proj_v_copy(j + 1, vps))
                # spread fillers across the quad positions
                npos = max(1, j - 1)
                base, extra = divmod(len(pe_filler), npos)
                for idx in range(2, j + 1):
                    take = base + (1 if idx - 2 < extra else 0)
                    for _ in range(take):
                        pe_filler.pop(0)()
                    qd = order[idx]
                    ps[qd] = sq(j, qd, q_sb)
                    emit_o(idx - 2)
                for f in pe_filler:
                    f()
                tail_start = max(2, j + 1) - 2
                if j < NSLOT - 1:
                    deferred = [
                        (lambda idx=idx, e=emit_o: e(idx))
                        for idx in range(tail_start, j + 1)
                    ]
                else:
                    for idx in range(tail_start, j + 1):
                        emit_o(idx)

    nc.compile()
    return nc


def kernel(X, Wq, Wk, Wv):
    global _PROGRAM
    X = np.asarray(X, dtype=np.float32)
    Wq = np.asarray(Wq, dtype=np.float32)
    Wk = np.asarray(Wk, dtype=np.float32)
    Wv = np.asarray(Wv, dtype=np.float32)

    if _PROGRAM is None:
        _PROGRAM = _build_program()
    nc = _PROGRAM

    wqk_cm = np.concatenate([Wq, Wk], axis=1).astype(XNP)  # [C, 128]
    wqk = np.ascontiguousarray(
        wqk_cm.reshape(CCH, 128, 128).transpose(1, 0, 2).reshape(128, CCH * 128)
    )
    wv = np.ascontiguousarray(
        Wv.astype(XNP).reshape(CCH, 128, 64).transpose(1, 0, 2).reshape(128, CCH * 64)
    )
    kp = np.arange(128, dtype=np.float32).reshape(128, 1)

    in_maps = []
    for core in range(NCORES):
        b, half = core // 2, core % 2
        in_maps.append(
            {
                "xt": np.ascontiguousarray(X[b].T).astype(XNP),
                "wqk": wqk,
                "wv": wv,
                "tau": kp - 256.0 * half,
                "sel": np.ascontiguousarray(
                    np.broadcast_to(
                        np.asarray([1.0 - half, float(half)], np.float32), (64, 2)
                    )
                ),
            }
        )

    trace = bool(os.environ.get("KERNEL_TRACE"))
    if trace:
        try:
            from antenv.axon_hooks import get_axon_ntff_profile_hook  # noqa: F401
        except ImportError:
            print(
                "KERNEL_TRACE requested but axon NTFF hook unavailable; running untraced"
            )
            trace = False
    kwargs = {}
    if trace:
        kwargs = dict(
            trace=True,
            trace_cores=[
                int(c) for c in os.environ.get("KERNEL_TRACE_CORES", "0").split(",")
            ],
        )
    res = run_bass_kernel_spmd(nc, in_maps, core_ids=list(range(NCORES)), **kwargs)
    if trace:
        print(f"HW exec time: {res.exec_time_ns} ns")
        print(f"mean exec time: {res.mean_exec_time_ns} ns")
        kernel.last_results = res

    out = np.empty((B, T, H), dtype=np.float32)
    for core in range(NCORES):
        b, half = core // 2, core % 2
        oc = res.results[core]["o"].reshape(NSLOT, QB, 65)
        norm = oc[:, :, 0:64] / oc[:, :, 64:65]
        out[b].reshape(NSLOT, 2, QB, H)[:, half] = norm
    return out
